# revision 22
# baseline (speedup 1.0000x reference)
"""Trainium2 Bass kernel for nn_AR_decoder (autoregressive LSTM decoder).

Contract: kernel(**inputs) takes FULL unsharded numpy inputs and returns the
FULL output [256, 2048, 5] f32 (per-step log_softmax of the decoder).

Strategy (hardcoded, self-contained):
  - Data-parallel: B=256 sharded across 8 NeuronCores (32 rows each); the
    sequential scan over T=2048 runs locally per shard; tiny params replicated.
  - Batch lives on SBUF partitions everywhere; per step the LSTM is a chain of
    small PSUM-accumulating matmuls in [b, j] layout:
        gates[32(b),128(j)] = x_t*W_x + h*W_hh + onehot*(W_p@emb.T) + 1*b
    The argmax-embedding feedback folds into one matmul by precomputing
    Wpe = W_p @ emb.T on the host (exact: onehot row selection).
  - Raw Bass (no Tile): engine streams with hand-counted semaphore waits;
    cross-engine deps form a PE->ACT->DVE ring per step. Standalone wait
    instructions avoid the 1-wait-per-matmul walrus limit.
  - log_softmax is deferred: the loop stores raw logits + row max; a
    vectorized post-pass computes logp = z - ln(sum exp z), z = logits - max.
"""

import os
import numpy as np
from contextlib import ExitStack

import concourse.bass as bass
from concourse import mybir
from concourse.bass_utils import run_bass_kernel_spmd

B, T, IN, H, NCLS = 256, 2048, 64, 32, 5
NCORES = 8
BL = B // NCORES  # 32 batch rows per core

AF = mybir.ActivationFunctionType
ALU = mybir.AluOpType
AX = mybir.AxisListType
F32 = mybir.dt.float32

_TT = int(os.environ.get("KB_TT", T))
_TRACE = os.environ.get("KB_TRACE", "0") == "1"

LAST_EXEC_NS = None
LAST_RESULTS = None


def _build(TT: int):
    THALF = TT // 2
    nc = bass.Bass()

    x_d = nc.declare_dram_parameter("xT", [128, THALF * BL], F32, isOutput=False)
    wx_d = nc.declare_dram_parameter("wxT", [2 * IN, 4 * H], F32, isOutput=False)
    wpe_d = nc.declare_dram_parameter("wpeT", [NCLS, 4 * H], F32, isOutput=False)
    whh_d = nc.declare_dram_parameter("whhT", [H, 4 * H], F32, isOutput=False)
    wfc_d = nc.declare_dram_parameter("wfcT", [H, NCLS], F32, isOutput=False)
    b_d = nc.declare_dram_parameter("brow", [1, 4 * H + BL + NCLS], F32,
                                    isOutput=False)
    out_d = nc.declare_dram_parameter("out", [BL, TT * NCLS], F32, isOutput=True)

    N_XCHUNK = 16 if TT >= 256 else 1
    XCOLS = THALF * BL
    CCOLS = XCOLS // N_XCHUNK
    TPC = TT // N_XCHUNK                 # timesteps per x chunk

    N_LCHUNK = 8 if TT >= 256 else 1
    LT = TT // N_LCHUNK                  # timesteps per logp chunk

    # ---- semaphore tick formulas (every compute instr increments its sem) ---
    DVE_INIT = 4                                    # 4 init memsets
    dve = lambda t, k: DVE_INIT + 9 * t + k         # k=1..9 per step
    pe = lambda t, k: 6 * t + k                     # k=1..6 per step
    act = lambda t, k: 3 * t + k                    # k=1..3 per step
    DVE_LOOP_END = DVE_INIT + 9 * TT
    ACT_LOOP_END = 3 * TT
    dve_post = lambda c, k: DVE_LOOP_END + 3 * c + k   # sub/rsum/fsub
    act_post = lambda c, k: ACT_LOOP_END + 2 * c + k   # exp/ln

    with ExitStack() as ctx:
        def sb(name, shape):
            return ctx.enter_context(nc.sbuf_tensor(name, shape, F32))

        xt = sb("xt_sb", [128, XCOLS])
        ls = sb("ls", [BL, TT * NCLS])
        ms = sb("ms", [BL, TT])
        oh = sb("oh", [BL, 32])
        wx = sb("wx", [2 * IN, 4 * H])
        wpe = sb("wpe", [NCLS, 4 * H])
        whh = sb("whh", [H, 4 * H])
        wfc = sb("wfc", [H, NCLS])
        brow = sb("brow_sb", [1, 4 * H + BL + NCLS])
        sifo = sb("sifo", [BL, 96])
        tg = sb("tg", [BL, 32])
        ig = sb("ig", [BL, 32])
        fcp = sb("fcp", [BL, 32])
        cc = sb("cc", [BL, 32])
        tcc = sb("tcc", [BL, 32])
        hb = sb("hb", [BL, 32])
        ht = sb("ht", [32, BL])
        oht = sb("oht", [32, 32])
        et = sb("et", [BL, LT * NCLS])
        se = sb("se", [BL, LT])
        lse = sb("lse", [BL, LT])

        bia = brow[:, 0:4 * H]
        onesr = brow[:, 4 * H:4 * H + BL]
        bfc = brow[:, 4 * H + BL:]

        g0 = ctx.enter_context(nc.psum_tensor("g0", [BL, 512], F32))
        g1 = ctx.enter_context(nc.psum_tensor("g1", [BL, 512], F32))
        l0 = ctx.enter_context(nc.psum_tensor("l0", [BL, 512], F32))
        l1 = ctx.enter_context(nc.psum_tensor("l1", [BL, 512], F32))
        gb = [g0, g1]
        lb = [l0, l1]

        s_dmaw = ctx.enter_context(nc.semaphore("s_dmaw"))
        s_dmax = ctx.enter_context(nc.semaphore("s_dmax"))
        s_dmao = ctx.enter_context(nc.semaphore("s_dmao"))
        s_pe = ctx.enter_context(nc.semaphore("s_pe"))
        s_act = ctx.enter_context(nc.semaphore("s_act"))
        s_dve = ctx.enter_context(nc.semaphore("s_dve"))

        with nc.Block() as block:

            @block.sync
            def _(sync):
                for dst, src in ((wx, wx_d), (wpe, wpe_d), (whh, whh_d),
                                 (wfc, wfc_d), (brow, b_d)):
                    sync.dma_start(dst[:], src[:]).then_inc(s_dmaw, 16)
                for c in range(N_XCHUNK):
                    sync.dma_start(
                        xt[:, c * CCOLS:(c + 1) * CCOLS],
                        x_d[:, c * CCOLS:(c + 1) * CCOLS],
                    ).then_inc(s_dmax, 16)
                for c in range(N_LCHUNK):
                    sync.wait_ge(s_dve, dve_post(c, 3))
                    sync.dma_start(
                        out_d[:, c * LT * NCLS:(c + 1) * LT * NCLS],
                        ls[:, c * LT * NCLS:(c + 1) * LT * NCLS],
                    ).then_inc(s_dmao, 16)
                sync.wait_ge(s_dmao, 16 * N_LCHUNK)

            @block.tensor
            def _(pe_e):
                for t in range(TT):
                    g = gb[t % 2]
                    if t == 0:
                        pe_e.wait_ge(s_dmaw, 80)
                        pe_e.wait_ge(s_dve, DVE_INIT)
                    else:
                        pe_e.wait_ge(s_dve, dve(t - 1, 5))   # ht ready
                    nc.tensor.matmul(g[:, 0:128], ht[:], whh[:],
                                     start=True, stop=False).then_inc(s_pe)
                    nc.tensor.matmul(g[:, 0:128], onesr, bia,
                                     start=False, stop=False).then_inc(s_pe)
                    if t % TPC == 0:
                        pe_e.wait_ge(s_dmax, 16 * (t // TPC + 1))
                    poff = (t % 2) * 64
                    col = (t // 2) * BL
                    nc.tensor.matmul(g[:, 0:128],
                                     xt[poff:poff + 64, col:col + BL],
                                     wx[poff:poff + 64, :],
                                     start=False, stop=False).then_inc(s_pe)
                    if t > 0:
                        pe_e.wait_ge(s_dve, dve(t - 1, 8))   # oht ready
                    nc.tensor.matmul(g[:, 0:128], oht[0:NCLS, :], wpe[:],
                                     start=False, stop=True).then_inc(s_pe)
                    lg = lb[t % 2]
                    pe_e.wait_ge(s_dve, dve(t, 5))           # ht_t ready
                    nc.tensor.matmul(lg[:, 0:NCLS], ht[:], wfc[:],
                                     start=True, stop=False).then_inc(s_pe)
                    nc.tensor.matmul(lg[:, 0:NCLS], onesr, bfc,
                                     start=False, stop=True).then_inc(s_pe)

            @block.scalar
            def _(act_e):
                for t in range(TT):
                    g = gb[t % 2]
                    act_e.wait_ge(s_pe, pe(t, 4))            # gates group done
                    nc.scalar.activation(sifo[:], g[:, 0:96],
                                         AF.Sigmoid).then_inc(s_act)
                    nc.scalar.activation(tg[:], g[:, 96:128],
                                         AF.Tanh).then_inc(s_act)
                    act_e.wait_ge(s_dve, dve(t, 3))          # c updated
                    nc.scalar.activation(tcc[:], cc[:],
                                         AF.Tanh).then_inc(s_act)
                for c in range(N_LCHUNK):
                    csl = slice(c * LT * NCLS, (c + 1) * LT * NCLS)
                    act_e.wait_ge(s_dve, dve_post(c, 1))
                    nc.scalar.activation(et[:], ls[:, csl],
                                         AF.Exp).then_inc(s_act)
                    act_e.wait_ge(s_dve, dve_post(c, 2))
                    nc.scalar.activation(lse[:], se[:], AF.Ln).then_inc(s_act)

            @block.vector
            def _(dve_e):
                nc.vector.memset(oh[:], 0.0).then_inc(s_dve)
                nc.vector.memset(ht[:], 0.0).then_inc(s_dve)
                nc.vector.memset(oht[:], 0.0).then_inc(s_dve)
                nc.vector.memset(cc[:], 0.0).then_inc(s_dve)
                for t in range(TT):
                    lg = lb[t % 2]
                    dve_e.wait_ge(s_act, act(t, 2))          # sig+tanh done
                    nc.vector.tensor_mul(ig[:], sifo[:, 0:32],
                                         tg[:]).then_inc(s_dve)
                    nc.vector.tensor_mul(fcp[:], sifo[:, 32:64],
                                         cc[:]).then_inc(s_dve)
                    nc.vector.tensor_add(cc[:], fcp[:], ig[:]).then_inc(s_dve)
                    dve_e.wait_ge(s_act, act(t, 3))          # tanh(c) done
                    nc.vector.tensor_mul(hb[:], sifo[:, 64:96],
                                         tcc[:]).then_inc(s_dve)
                    dve_e.drain()
                    nc.vector.transpose(ht[:], hb[:]).then_inc(s_dve)
                    dve_e.wait_ge(s_pe, pe(t, 6))            # logits done
                    nc.vector.reduce_max(ms[:, t:t + 1], lg[:, 0:NCLS],
                                         axis=AX.X).then_inc(s_dve)
                    dve_e.drain()
                    nc.vector.tensor_scalar(oh[:, 0:NCLS], lg[:, 0:NCLS],
                                            ms[:, t:t + 1], None,
                                            ALU.is_equal).then_inc(s_dve)
                    dve_e.drain()
                    nc.vector.transpose(oht[:], oh[:]).then_inc(s_dve)
                    nc.vector.tensor_copy(ls[:, t * NCLS:(t + 1) * NCLS],
                                          lg[:, 0:NCLS]).then_inc(s_dve)
                for c in range(N_LCHUNK):
                    csl = slice(c * LT * NCLS, (c + 1) * LT * NCLS)
                    z3 = ls[:, csl].rearrange("p (t c) -> p t c", c=NCLS)
                    mb = ms[:, c * LT:(c + 1) * LT].broadcast_to([BL, LT, NCLS])
                    nc.vector.tensor_tensor(z3, z3, mb,
                                            ALU.subtract).then_inc(s_dve)
                    dve_e.wait_ge(s_act, act_post(c, 1))     # exp done
                    nc.vector.reduce_sum(
                        se[:], et[:].rearrange("p (t c) -> p t c", c=NCLS),
                        axis=AX.X).then_inc(s_dve)
                    dve_e.wait_ge(s_act, act_post(c, 2))     # ln done
                    lseb = lse[:].broadcast_to([BL, LT, NCLS])
                    nc.vector.tensor_tensor(z3, z3, lseb,
                                            ALU.subtract).then_inc(s_dve)

    return nc


def _prep(x, W_ih, W_hh, b_ih, b_hh, W_fc, b_fc, emb, TT):
    """Host-side layout prep. Returns per-core input maps."""
    x = np.asarray(x, dtype=np.float32)
    W_ih = np.asarray(W_ih, dtype=np.float32)
    W_hh = np.asarray(W_hh, dtype=np.float32)
    b = (np.asarray(b_ih, dtype=np.float32) + np.asarray(b_hh, dtype=np.float32))
    W_fc = np.asarray(W_fc, dtype=np.float32)
    b_fc = np.asarray(b_fc, dtype=np.float32)
    emb = np.asarray(emb, dtype=np.float32)

    # permute PyTorch gate rows [i, f, g, o] -> [i, f, o, g]
    perm = np.concatenate([np.arange(0, 64), np.arange(96, 128),
                           np.arange(64, 96)])
    W_ih_p = W_ih[perm]
    W_hh_p = W_hh[perm]
    b_p = b[perm]

    W_x = W_ih_p[:, :IN]                      # [128, 64]
    W_p = W_ih_p[:, IN:]                      # [128, 64]
    Wpe = W_p @ emb.T                         # [128, 5]

    wxT = np.ascontiguousarray(np.vstack([W_x.T, W_x.T]))  # [128, 128] dup
    wpeT = np.ascontiguousarray(Wpe.T)        # [5, 128]
    whhT = np.ascontiguousarray(W_hh_p.T)     # [32, 128]
    wfcT = np.ascontiguousarray(W_fc.T)       # [32, 5]
    brow = np.ascontiguousarray(
        np.concatenate([b_p, np.ones(BL, np.float32), b_fc]).reshape(1, -1))

    in_maps = []
    for ci in range(NCORES):
        xs = x[ci * BL:(ci + 1) * BL, :TT]    # [BL, TT, 64]
        # -> [128, (TT//2)*BL]; partition p=(t%2)*64+f, free col=(t//2)*BL+b
        y = xs.transpose(1, 2, 0)             # [TT, F, B]
        y = y.reshape(TT // 2, 2, IN, BL).transpose(1, 2, 0, 3)
        y = np.ascontiguousarray(y.reshape(128, (TT // 2) * BL))
        in_maps.append({
            "xT": y, "wxT": wxT, "wpeT": wpeT, "whhT": whhT,
            "wfcT": wfcT, "brow": brow,
        })
    return in_maps


def kernel(x, x_lengths, edge_list, W_ih, W_hh, b_ih, b_hh, W_fc, b_fc, emb):
    global LAST_EXEC_NS, LAST_RESULTS
    TT = _TT
    inputs = _prep(x, W_ih, W_hh, b_ih, b_hh, W_fc, b_fc, emb, TT)

    nc = _build(TT)
    res = run_bass_kernel_spmd(
        nc, inputs, core_ids=list(range(NCORES)), trace=_TRACE,
    )
    LAST_EXEC_NS = res.exec_time_ns
    LAST_RESULTS = res

    outs = [res.results[i]["out"].reshape(BL, TT, NCLS) for i in range(NCORES)]
    full = np.concatenate(outs, axis=0)
    if TT < T:
        pad = np.zeros((B, T - TT, NCLS), dtype=np.float32)
        full = np.concatenate([full, pad], axis=1)
    return full
